# revision 30
# baseline (speedup 1.0000x reference)
"""Trainium2 Bass kernel for nn_AttnBlockpp3d_old (GroupNorm + 4-head spatial
self-attention + residual), data-parallel over batch across 8 NeuronCores.

Shapes (hardcoded): x [16, 256, 32, 32] f32, 4 nin weights [256, 256] + biases,
gn scale/bias [256]. Each core processes 2 batches of [256, 1024].

Structure (per core): phase 1 runs GroupNorm stats + q/k/v projections for
BOTH batches up front; phase 2 runs the attention pairs + final nin
back-to-back so the ScalarE softmax-exp stream (the bottleneck engine) is
continuous and the PE stays HAM-warm.

Key tricks:
- GroupNorm stats via bn_stats on a contiguous [128, 2048] view (partition
  4g+j holds 2 channels of group g); group-combine and channel-broadcast via
  tiny indicator matmuls; rsqrt as exp(-0.5*ln(var+eps)) so ScalarE stays on
  the ln+exp table set used by the softmax.
- v is produced directly transposed (h slices stationary, W2 moving): no PE
  transposes anywhere.
- Scores are computed transposed ([t, s], k stationary) with two heads packed
  into the PE array via row tiling (64-partition contraction each).
- Softmax exp on ScalarE straight out of PSUM with the 1/sqrt(64) scale
  folded into the activation; no max-subtraction (scores are O(+-7)).
- The softmax denominator rides the A@V matmul as a ones-column in the
  stationary operand; normalization = reciprocal_approx_fast + DRAM-bounce
  partition-broadcast DMA, fused into the mandatory PSUM->SBUF move.
- Final nin adds b3 via a K=1 matmul; the residual rides the PSUM->SBUF move.
"""

import numpy as np

N_CORES = 8
B_TOTAL = 16
B_PER_CORE = B_TOTAL // N_CORES
C = 256
H = 32
S = H * H          # 1024 spatial positions (N_FRAMES=1)
NG = 32            # groupnorm groups -> 8 channels/group
NH = 4             # heads
CH = C // NH       # 64 channels/head
EPS = 1e-6
SCALE = CH ** -0.5  # 0.125

_CACHE: dict = {}


def _build_nc(debug_taps=False):
    from contextlib import ExitStack

    import concourse.bacc as bacc
    import concourse.bass as bass
    import concourse.mybir as mybir
    import concourse.tile as tile

    fp32 = mybir.dt.float32
    bf16 = mybir.dt.bfloat16
    AF = mybir.ActivationFunctionType
    OP = mybir.AluOpType
    ts = bass.ts

    nc = bacc.Bacc("TRN2")

    x_d = nc.dram_tensor("x", [B_PER_CORE, C, S], fp32, kind="ExternalInput")
    gns_d = nc.dram_tensor("gn_scale", [C], fp32, kind="ExternalInput")
    gnb_d = nc.dram_tensor("gn_bias", [C], fp32, kind="ExternalInput")
    W_d = [nc.dram_tensor(f"W{i}", [C, C], fp32, kind="ExternalInput") for i in range(4)]
    b_d = [nc.dram_tensor(f"b{i}", [C], fp32, kind="ExternalInput") for i in range(4)]
    y_d = nc.dram_tensor("y", [B_PER_CORE, C, S], fp32, kind="ExternalOutput")
    dbg = {}
    if debug_taps:
        for nm, shp, dt_ in (("h", [2, 128, S], bf16), ("q", [2, 128, S], bf16),
                             ("k", [2, 128, S], bf16), ("vt0", [128, NH, CH + 1], bf16),
                             ("e00", [128, S], bf16), ("rd20", [1, S], fp32),
                             ("hh0", [128, S], bf16), ("ab", [2, 128, 2], fp32)):
            dbg[nm] = nc.dram_tensor(f"dbg_{nm}", shp, dt_, kind="ExternalOutput")

    with tile.TileContext(nc) as tc, ExitStack() as ctx:
        const = ctx.enter_context(tc.tile_pool(name="const", bufs=1))
        stage = ctx.enter_context(tc.tile_pool(name="stage", bufs=2))
        xpool = ctx.enter_context(tc.tile_pool(name="xpool", bufs=2))
        hpool = ctx.enter_context(tc.tile_pool(name="hpool", bufs=2))
        vpool = ctx.enter_context(tc.tile_pool(name="vpool", bufs=18))
        epool = ctx.enter_context(tc.tile_pool(name="epool", bufs=6))
        rpool = ctx.enter_context(tc.tile_pool(name="rpool", bufs=2))
        dpool = ctx.enter_context(tc.tile_pool(name="dpool", bufs=4, space="DRAM"))
        spool = ctx.enter_context(tc.tile_pool(name="spool", bufs=3))

        # PSUM (8 banks): T0/T1 = 2-bank slots (hh accumulators / qkv / fin),
        # s0/s1 = 1-bank slots x2 bufs (scores double-buffer / vt / stats).
        ps = ctx.enter_context(tc.tile_pool(name="ps", bufs=1, space="PSUM"))

        # ---- phase 0: loads + constants ----
        # x loads first (stats are on the critical path)
        xs = []
        for b in range(B_PER_CORE):
            x_sb = []
            for ct in range(2):
                t = xpool.tile([128, S], fp32, tag=f"x{b}{ct}", name=f"x_sb{b}{ct}")
                nc.sync.dma_start(out=t, in_=x_d[b, ts(ct, 128), :])
                x_sb.append(t)
            xg = xpool.tile([128, 2 * S], fp32, tag="xg")
            nc.sync.dma_start(out=xg, in_=x_d[b].rearrange("(p a) s -> p (a s)", p=128))
            xs.append((x_sb, xg))

        # W0..W3 as bf16 [128, c_tile 2, d 256] (partition p = channel p + 128*ct)
        Wsb_t = []
        for i in range(4):
            st = stage.tile([128, 2, C], fp32, tag="wstage")
            nc.sync.dma_start(out=st, in_=W_d[i].rearrange("(a p) d -> p a d", p=128))
            wt = const.tile([128, 2, C], bf16, tag=f"w{i}")
            nc.gpsimd.tensor_copy(out=wt, in_=st)
            Wsb_t.append(wt)
        Wsb = [[Wsb_t[i][:, ct, :] for ct in range(2)] for i in range(4)]

        def col_tiles(dram, name):
            out = []
            for ct in range(2):
                t = const.tile([128, 1], fp32, tag=f"{name}{ct}")
                nc.sync.dma_start(out=t, in_=dram[ts(ct, 128)][:, None])
                out.append(t)
            return out

        gns_sb = col_tiles(gns_d, "gns")
        gnb_sb = col_tiles(gnb_d, "gnb")
        b0_sb = col_tiles(b_d[0], "b0")
        b1_sb = col_tiles(b_d[1], "b1")

        b2b = const.tile([128, C], fp32, tag="b2b")
        nc.sync.dma_start(out=b2b, in_=b_d[2][None, :].to_broadcast([128, C]))

        b3_sb = col_tiles(b_d[3], "b3")

        eps_t = const.tile([32, 1], fp32, tag="eps")
        nc.vector.memset(eps_t, EPS)

        # HAM warm-up: dummy matmuls with no data deps keep the PE busy during
        # the load phase so real matmuls start at the unthrottled clock.
        warm = const.tile([128, 512], bf16, tag="warm")
        nc.vector.memset(warm, 1.0)
        warm_ps = ps.tile([128, 512], fp32, tag="s0", bufs=2, name="warm_ps")
        for i in range(40):
            nc.tensor.matmul(warm_ps, lhsT=warm[:, 0:128], rhs=warm,
                             start=True, stop=True)

        # Q1 [128, 32]: Q1[p, g] = 1 iff p//4 == g   (stats partition -> group)
        q1 = const.tile([128, NG], fp32, tag="q1")
        nc.gpsimd.memset(q1, 1.0)
        nc.gpsimd.affine_select(out=q1, in_=q1, compare_op=OP.is_ge, fill=0.0,
                                pattern=[[-4, NG]], base=0, channel_multiplier=1)
        nc.gpsimd.affine_select(out=q1, in_=q1, compare_op=OP.is_ge, fill=0.0,
                                pattern=[[4, NG]], base=3, channel_multiplier=-1)

        # Q2[ct] [32, 128]: Q2[g, c] = 1 iff group(global_c) == g
        q2 = []
        for ct in range(2):
            t = const.tile([NG, 128], fp32, tag=f"q2{ct}")
            nc.gpsimd.memset(t, 1.0)
            base = ct * 128
            nc.gpsimd.affine_select(out=t, in_=t, compare_op=OP.is_ge, fill=0.0,
                                    pattern=[[1, 128]], base=base, channel_multiplier=-8)
            nc.gpsimd.affine_select(out=t, in_=t, compare_op=OP.is_ge, fill=0.0,
                                    pattern=[[-1, 128]], base=7 - base, channel_multiplier=8)
            q2.append(t)

        # ---- phase 1 per batch: stats + normalize + q/k/vT ----
        qk_all, vt_all = [], []
        for b in range(B_PER_CORE):
            x_sb, xg = xs[b]
            st6 = spool.tile([128, 4, 6], fp32, tag="st6")
            for i in range(4):
                nc.vector.bn_stats(out=st6[:, i, :], in_=xg[:, ts(i, 512)])
            mv = spool.tile([128, 2], fp32, tag="mv")
            nc.vector.bn_aggr(out=mv, in_=st6)
            rhs2 = spool.tile([128, 2], fp32, tag="rhs2")
            nc.vector.tensor_copy(out=rhs2[:, 0:1], in_=mv[:, 0:1])
            nc.vector.tensor_mul(out=rhs2[:, 1:2], in0=mv[:, 0:1], in1=mv[:, 0:1])
            nc.vector.tensor_add(out=rhs2[:, 1:2], in0=rhs2[:, 1:2], in1=mv[:, 1:2])
            gs_ps = ps.tile([NG, 2], fp32, tag="m0")
            nc.tensor.matmul(gs_ps, lhsT=q1, rhs=rhs2, start=True, stop=True)
            gmv = spool.tile([NG, 2], fp32, tag="gmv")
            nc.vector.tensor_scalar_mul(out=gmv, in0=gs_ps, scalar1=0.25)
            varg = spool.tile([NG, 1], fp32, tag="varg")
            nc.vector.tensor_mul(out=varg, in0=gmv[:, 0:1], in1=gmv[:, 0:1])
            nc.vector.tensor_tensor(out=varg, in0=gmv[:, 1:2], in1=varg,
                                    op=OP.subtract)
            ab_g = spool.tile([NG, 2], fp32, tag="abg")
            lnv = spool.tile([NG, 1], fp32, tag="lnv")
            nc.scalar.activation(out=lnv, in_=varg, func=AF.Ln, bias=eps_t, scale=1.0)
            nc.scalar.activation(out=ab_g[:, 0:1], in_=lnv, func=AF.Exp, scale=-0.5)
            nc.vector.tensor_mul(out=ab_g[:, 1:2], in0=gmv[:, 0:1], in1=ab_g[:, 0:1])
            nc.vector.tensor_scalar_mul(out=ab_g[:, 1:2], in0=ab_g[:, 1:2],
                                        scalar1=-1.0)

            h_bf = []
            for ct in range(2):
                ab_ps = ps.tile([128, 2], fp32, tag="m1")
                nc.tensor.matmul(ab_ps, lhsT=q2[ct], rhs=ab_g, start=True, stop=True)
                AB = spool.tile([128, 2], fp32, tag=f"AB{ct}")
                nc.vector.tensor_mul(out=AB[:, 0:1], in0=ab_ps[:, 0:1], in1=gns_sb[ct])
                nc.vector.tensor_mul(out=AB[:, 1:2], in0=ab_ps[:, 1:2], in1=gns_sb[ct])
                nc.vector.tensor_add(out=AB[:, 1:2], in0=AB[:, 1:2], in1=gnb_sb[ct])
                ht = hpool.tile([128, S], bf16, tag=f"h{ct}")
                nc.vector.tensor_scalar(out=ht, in0=x_sb[ct],
                                        scalar1=AB[:, 0:1], scalar2=AB[:, 1:2],
                                        op0=OP.mult, op1=OP.add)
                if debug_taps and b == 0:
                    nc.sync.dma_start(out=dbg["h"][ct], in_=ht)
                    nc.sync.dma_start(out=dbg["ab"][ct], in_=AB)
                h_bf.append(ht)
            # residual tile absorbs b3 (x + b3 + W3 hh is the final output)
            for ct in range(2):
                nc.vector.tensor_scalar_add(out=x_sb[ct], in0=x_sb[ct],
                                            scalar1=b3_sb[ct])

            # q/k projections -> bf16 [d_tile 128, s 1024]
            qk_sb = [[None, None], [None, None]]
            for p, bias in ((0, b0_sb), (1, b1_sb)):
                for dt in range(2):
                    t = hpool.tile([128, S], bf16, tag=f"qk{p}{dt}")
                    for sc in range(2):
                        qk_ps = ps.tile([128, 512], fp32, tag=f"m{sc}",
                                        name="qk_ps")
                        for ct in range(2):
                            nc.tensor.matmul(
                                qk_ps,
                                lhsT=Wsb[p][ct][:, ts(dt, 128)],
                                rhs=h_bf[ct][:, ts(sc, 512)],
                                start=(ct == 0), stop=(ct == 1))
                        nc.vector.tensor_scalar_add(out=t[:, ts(sc, 512)],
                                                    in0=qk_ps, scalar1=bias[dt])
                    if debug_taps and b == 0:
                        nc.sync.dma_start(out=dbg["q" if p == 0 else "k"][dt], in_=t)
                    qk_sb[p][dt] = t
            qk_all.append(qk_sb)

            # v, produced transposed: vT[t, d] with ones column per head
            vt_tiles = []
            for j in range(8):
                vt_ps = ps.tile([128, C], fp32, tag=f"m{j % 2}", name="vt_ps")
                for ct in range(2):
                    nc.tensor.matmul(vt_ps, lhsT=h_bf[ct][:, ts(j, 128)],
                                     rhs=Wsb[2][ct], start=(ct == 0), stop=(ct == 1))
                vt = vpool.tile([128, NH, CH + 1], bf16, tag="vt")
                nc.gpsimd.memset(vt[:, :, CH:CH + 1], 1.0)
                nc.vector.tensor_tensor(
                    out=vt[:, :, 0:CH],
                    in0=vt_ps.rearrange("p (h c) -> p h c", h=NH),
                    in1=b2b.rearrange("p (h c) -> p h c", h=NH),
                    op=OP.add)
                if debug_taps and b == 0 and j == 0:
                    nc.sync.dma_start(out=dbg["vt0"][:, :, :], in_=vt)
                vt_tiles.append(vt)
            vt_all.append(vt_tiles)

        # ---- phase 2 per batch: attention pairs + final nin ----
        for b in range(B_PER_CORE):
            x_sb, _ = xs[b]
            qk_sb = qk_all[b]
            vt_tiles = vt_all[b]
            hh_sb = [hpool.tile([128, S], bf16, tag="hh", bufs=4,
                                 name=f"hh_t{i}") for i in range(2)]
            out_ts = [xpool.tile([128, S], fp32, tag=f"out{dt}",
                                 name=f"out_t{dt}") for dt in range(2)]
            for sc in range(2):
                for pr in range(2):
                    hh_ps = [ps.tile([CH + 1, 512], fp32, tag=f"h{i}",
                                     name=f"hh_ps{i}") for i in range(2)]
                    for j in range(8):
                        for hp in range(2):
                            s_ps = ps.tile([128, 512], fp32, tag=f"s{hp}",
                                           bufs=2, name="s_ps")
                            nc.tensor.matmul(
                                s_ps,
                                lhsT=qk_sb[1][pr][ts(hp, CH), ts(j, 128)],
                                rhs=qk_sb[0][pr][ts(hp, CH), ts(sc, 512)],
                                start=True, stop=True)
                            et = epool.tile([128, 512], bf16, tag="e")
                            nc.scalar.activation(out=et, in_=s_ps,
                                                 func=AF.Exp, scale=SCALE)
                            if debug_taps and b == 0 and pr == 0 and j == 0 and hp == 0:
                                nc.sync.dma_start(out=dbg["e00"][:, ts(sc, 512)], in_=et)
                            nc.tensor.matmul(
                                hh_ps[hp],
                                lhsT=vt_tiles[j][:, 2 * pr + hp, :],
                                rhs=et,
                                start=(j == 0), stop=(j == 7))
                    # per-half tail: denominators for these s columns are final
                    for hp in range(2):
                        hh_u = rpool.tile([CH + 1, 512], fp32, tag=f"hhu{hp}",
                                          bufs=4, name="hh_u")
                        nc.vector.tensor_copy(out=hh_u, in_=hh_ps[hp])
                        rd2 = rpool.tile([CH + 1, 512], fp32, tag="rd2",
                                         bufs=4, name="rd2")
                        nc.vector.reciprocal_approx_fast(out=rd2, in_=hh_u)
                        if debug_taps and b == 0 and pr == 0 and hp == 0:
                            nc.sync.dma_start(out=dbg["rd20"][:, ts(sc, 512)],
                                              in_=rd2[CH:CH + 1, :])
                        rdd = dpool.tile([1, 512], fp32, tag="rdd")
                        nc.gpsimd.dma_start(out=rdd, in_=rd2[CH:CH + 1, :])
                        rdb = rpool.tile([CH, 512], fp32, tag="rdb", bufs=4)
                        nc.gpsimd.dma_start(out=rdb, in_=rdd.to_broadcast([CH, 512]))
                        nc.vector.tensor_mul(
                            out=hh_sb[pr][ts(hp, CH), ts(sc, 512)],
                            in0=hh_u[0:CH, :], in1=rdb)
                # per-half final nin + residual
                for dt in range(2):
                    fin_ps = ps.tile([128, 512], fp32, tag=f"m{sc}",
                                     name="fin_ps")
                    for ct in range(2):
                        nc.tensor.matmul(
                            fin_ps,
                            lhsT=Wsb[3][ct][:, ts(dt, 128)],
                            rhs=hh_sb[ct][:, ts(sc, 512)],
                            start=(ct == 0), stop=(ct == 1))
                    nc.vector.tensor_add(out=out_ts[dt][:, ts(sc, 512)],
                                         in0=fin_ps,
                                         in1=x_sb[dt][:, ts(sc, 512)])
            if debug_taps and b == 0:
                nc.sync.dma_start(out=dbg["hh0"][:, :], in_=hh_sb[0])
            for dt in range(2):
                nc.gpsimd.dma_start(out=y_d[b, ts(dt, 128), :], in_=out_ts[dt])

    nc.finalize()
    return nc


def _in_maps(inputs):
    x = np.ascontiguousarray(np.asarray(inputs["x"], dtype=np.float32))
    B = x.shape[0]
    xr = x.reshape(B, C, S)
    shared = {k: np.ascontiguousarray(np.asarray(inputs[k], dtype=np.float32))
              for k in ("gn_scale", "gn_bias", "W0", "b0", "W1", "b1", "W2", "b2",
                        "W3", "b3")}
    maps = []
    for core in range(N_CORES):
        m = dict(shared)
        m["x"] = np.ascontiguousarray(xr[core * B_PER_CORE:(core + 1) * B_PER_CORE])
        maps.append(m)
    return maps


def kernel(**inputs: np.ndarray) -> np.ndarray:
    from concourse.bass_utils import run_bass_kernel_spmd

    if "nc" not in _CACHE:
        _CACHE["nc"] = _build_nc()
    res = run_bass_kernel_spmd(_CACHE["nc"], _in_maps(inputs),
                               core_ids=list(range(N_CORES)))
    out = np.concatenate([res.results[c]["y"] for c in range(N_CORES)], axis=0)
    B = np.asarray(inputs["x"]).shape[0]
    return out.reshape(B, C, H, H).astype(np.float32)


def run_profiled(inputs):
    """Like kernel() but with trace=True; returns (out, exec_time_ns)."""
    from concourse.bass_utils import run_bass_kernel_spmd

    if "nc" not in _CACHE:
        _CACHE["nc"] = _build_nc()
    res = run_bass_kernel_spmd(_CACHE["nc"], _in_maps(inputs),
                               core_ids=list(range(N_CORES)), trace=True)
    out = np.concatenate([res.results[c]["y"] for c in range(N_CORES)], axis=0)
    B = np.asarray(inputs["x"]).shape[0]
    return out.reshape(B, C, H, H).astype(np.float32), res.exec_time_ns


# revision 31
# speedup vs baseline: 1.1063x; 1.1063x over previous
"""Trainium2 Bass kernel for nn_AttnBlockpp3d_old (GroupNorm + 4-head spatial
self-attention + residual), data-parallel over batch across 8 NeuronCores.

Shapes (hardcoded): x [16, 256, 32, 32] f32, 4 nin weights [256, 256] + biases,
gn scale/bias [256]. Each core processes 2 batches of [256, 1024].

Structure (per core): phase 1 runs GroupNorm stats + q/k/v projections for
BOTH batches up front; phase 2 runs the attention pairs + final nin
back-to-back so the ScalarE softmax-exp stream (the bottleneck engine) is
continuous and the PE stays HAM-warm.

Key tricks:
- GroupNorm stats via bn_stats on a contiguous [128, 2048] view (partition
  4g+j holds 2 channels of group g); group-combine and channel-broadcast via
  tiny indicator matmuls; rsqrt as exp(-0.5*ln(var+eps)) so ScalarE stays on
  the ln+exp table set used by the softmax.
- v is produced directly transposed (h slices stationary, W2 moving): no PE
  transposes anywhere.
- Scores are computed transposed ([t, s], k stationary) with two heads packed
  into the PE array via row tiling (64-partition contraction each).
- Softmax exp on ScalarE straight out of PSUM with the 1/sqrt(64) scale
  folded into the activation; no max-subtraction (scores are O(+-7)).
- The softmax denominator rides the A@V matmul as a ones-column in the
  stationary operand; normalization = reciprocal_approx_fast + DRAM-bounce
  partition-broadcast DMA, fused into the mandatory PSUM->SBUF move.
- Final nin adds b3 via a K=1 matmul; the residual rides the PSUM->SBUF move.
"""

import numpy as np

N_CORES = 8
B_TOTAL = 16
B_PER_CORE = B_TOTAL // N_CORES
C = 256
H = 32
S = H * H          # 1024 spatial positions (N_FRAMES=1)
NG = 32            # groupnorm groups -> 8 channels/group
NH = 4             # heads
CH = C // NH       # 64 channels/head
EPS = 1e-6
SCALE = CH ** -0.5  # 0.125

_CACHE: dict = {}


def _build_nc(debug_taps=False):
    from contextlib import ExitStack

    import concourse.bacc as bacc
    import concourse.bass as bass
    import concourse.mybir as mybir
    import concourse.tile as tile

    fp32 = mybir.dt.float32
    bf16 = mybir.dt.bfloat16
    AF = mybir.ActivationFunctionType
    OP = mybir.AluOpType
    ts = bass.ts

    nc = bacc.Bacc("TRN2")

    x_d = nc.dram_tensor("x", [B_PER_CORE, C, S], fp32, kind="ExternalInput")
    gns_d = nc.dram_tensor("gn_scale", [C], fp32, kind="ExternalInput")
    gnb_d = nc.dram_tensor("gn_bias", [C], fp32, kind="ExternalInput")
    W_d = [nc.dram_tensor(f"W{i}", [C, C], fp32, kind="ExternalInput") for i in range(4)]
    b_d = [nc.dram_tensor(f"b{i}", [C], fp32, kind="ExternalInput") for i in range(4)]
    y_d = nc.dram_tensor("y", [B_PER_CORE, C, S], fp32, kind="ExternalOutput")
    dbg = {}
    if debug_taps:
        for nm, shp, dt_ in (("h", [2, 128, S], bf16), ("q", [2, 128, S], bf16),
                             ("k", [2, 128, S], bf16), ("vt0", [128, NH, CH + 1], bf16),
                             ("e00", [128, S], bf16), ("rd20", [1, S], fp32),
                             ("hh0", [128, S], bf16), ("ab", [2, 128, 2], fp32)):
            dbg[nm] = nc.dram_tensor(f"dbg_{nm}", shp, dt_, kind="ExternalOutput")

    with tile.TileContext(nc) as tc, ExitStack() as ctx:
        const = ctx.enter_context(tc.tile_pool(name="const", bufs=1))
        stage = ctx.enter_context(tc.tile_pool(name="stage", bufs=2))
        xpool = ctx.enter_context(tc.tile_pool(name="xpool", bufs=2))
        hpool = ctx.enter_context(tc.tile_pool(name="hpool", bufs=2))
        vpool = ctx.enter_context(tc.tile_pool(name="vpool", bufs=18))
        epool = ctx.enter_context(tc.tile_pool(name="epool", bufs=6))
        rpool = ctx.enter_context(tc.tile_pool(name="rpool", bufs=2))
        dpool = ctx.enter_context(tc.tile_pool(name="dpool", bufs=4, space="DRAM"))
        spool = ctx.enter_context(tc.tile_pool(name="spool", bufs=3))

        # PSUM (8 banks): T0/T1 = 2-bank slots (hh accumulators / qkv / fin),
        # s0/s1 = 1-bank slots x2 bufs (scores double-buffer / vt / stats).
        ps = ctx.enter_context(tc.tile_pool(name="ps", bufs=1, space="PSUM"))

        # ---- phase 0: loads + constants ----
        # x loads first (stats are on the critical path)
        xs = []
        for b in range(B_PER_CORE):
            x_sb = []
            for ct in range(2):
                t = xpool.tile([128, S], fp32, tag=f"x{b}{ct}", name=f"x_sb{b}{ct}")
                nc.sync.dma_start(out=t, in_=x_d[b, ts(ct, 128), :])
                x_sb.append(t)
            xg = xpool.tile([128, 2 * S], fp32, tag="xg")
            nc.sync.dma_start(out=xg, in_=x_d[b].rearrange("(p a) s -> p (a s)", p=128))
            xs.append((x_sb, xg))

        # W0..W3 as bf16 [128, c_tile 2, d 256] (partition p = channel p + 128*ct)
        Wsb_t = []
        for i in range(4):
            st = stage.tile([128, 2, C], fp32, tag="wstage")
            nc.sync.dma_start(out=st, in_=W_d[i].rearrange("(a p) d -> p a d", p=128))
            wt = const.tile([128, 2, C], bf16, tag=f"w{i}")
            nc.gpsimd.tensor_copy(out=wt, in_=st)
            Wsb_t.append(wt)
        Wsb = [[Wsb_t[i][:, ct, :] for ct in range(2)] for i in range(4)]

        def col_tiles(dram, name):
            out = []
            for ct in range(2):
                t = const.tile([128, 1], fp32, tag=f"{name}{ct}")
                nc.sync.dma_start(out=t, in_=dram[ts(ct, 128)][:, None])
                out.append(t)
            return out

        gns_sb = col_tiles(gns_d, "gns")
        gnb_sb = col_tiles(gnb_d, "gnb")
        b0_sb = col_tiles(b_d[0], "b0")
        b1_sb = col_tiles(b_d[1], "b1")

        b2b = const.tile([128, C], fp32, tag="b2b")
        nc.sync.dma_start(out=b2b, in_=b_d[2][None, :].to_broadcast([128, C]))

        b3_sb = col_tiles(b_d[3], "b3")

        eps_t = const.tile([32, 1], fp32, tag="eps")
        nc.vector.memset(eps_t, EPS)

        # HAM warm-up: dummy matmuls with no data deps keep the PE busy during
        # the load phase so real matmuls start at the unthrottled clock.
        warm = const.tile([128, 512], bf16, tag="warm")
        nc.vector.memset(warm, 1.0)
        warm_ps = ps.tile([128, 512], fp32, tag="s0", bufs=2, name="warm_ps")
        for i in range(40):
            nc.tensor.matmul(warm_ps, lhsT=warm[:, 0:128], rhs=warm,
                             start=True, stop=True)

        # Q1 [128, 32]: Q1[p, g] = 1 iff p//4 == g   (stats partition -> group)
        q1 = const.tile([128, NG], fp32, tag="q1")
        nc.gpsimd.memset(q1, 1.0)
        nc.gpsimd.affine_select(out=q1, in_=q1, compare_op=OP.is_ge, fill=0.0,
                                pattern=[[-4, NG]], base=0, channel_multiplier=1)
        nc.gpsimd.affine_select(out=q1, in_=q1, compare_op=OP.is_ge, fill=0.0,
                                pattern=[[4, NG]], base=3, channel_multiplier=-1)

        # Q2[ct] [32, 128]: Q2[g, c] = 1 iff group(global_c) == g
        q2 = []
        for ct in range(2):
            t = const.tile([NG, 128], fp32, tag=f"q2{ct}")
            nc.gpsimd.memset(t, 1.0)
            base = ct * 128
            nc.gpsimd.affine_select(out=t, in_=t, compare_op=OP.is_ge, fill=0.0,
                                    pattern=[[1, 128]], base=base, channel_multiplier=-8)
            nc.gpsimd.affine_select(out=t, in_=t, compare_op=OP.is_ge, fill=0.0,
                                    pattern=[[-1, 128]], base=7 - base, channel_multiplier=8)
            q2.append(t)

        # ---- phase 1 per batch: stats + normalize + q/k/vT ----
        qk_all, vt_all = [], []
        for b in range(B_PER_CORE):
            x_sb, xg = xs[b]
            st6 = spool.tile([128, 4, 6], fp32, tag="st6")
            for i in range(4):
                nc.vector.bn_stats(out=st6[:, i, :], in_=xg[:, ts(i, 512)])
            mv = spool.tile([128, 2], fp32, tag="mv")
            nc.vector.bn_aggr(out=mv, in_=st6)
            rhs2 = spool.tile([128, 2], fp32, tag="rhs2")
            nc.vector.tensor_copy(out=rhs2[:, 0:1], in_=mv[:, 0:1])
            nc.vector.tensor_mul(out=rhs2[:, 1:2], in0=mv[:, 0:1], in1=mv[:, 0:1])
            nc.vector.tensor_add(out=rhs2[:, 1:2], in0=rhs2[:, 1:2], in1=mv[:, 1:2])
            gs_ps = ps.tile([NG, 2], fp32, tag="m0")
            nc.tensor.matmul(gs_ps, lhsT=q1, rhs=rhs2, start=True, stop=True)
            gmv = spool.tile([NG, 2], fp32, tag="gmv")
            nc.vector.tensor_scalar_mul(out=gmv, in0=gs_ps, scalar1=0.25)
            varg = spool.tile([NG, 1], fp32, tag="varg")
            nc.vector.tensor_mul(out=varg, in0=gmv[:, 0:1], in1=gmv[:, 0:1])
            nc.vector.tensor_tensor(out=varg, in0=gmv[:, 1:2], in1=varg,
                                    op=OP.subtract)
            ab_g = spool.tile([NG, 2], fp32, tag="abg")
            lnv = spool.tile([NG, 1], fp32, tag="lnv")
            nc.scalar.activation(out=lnv, in_=varg, func=AF.Ln, bias=eps_t, scale=1.0)
            nc.scalar.activation(out=ab_g[:, 0:1], in_=lnv, func=AF.Exp, scale=-0.5)
            nc.vector.tensor_mul(out=ab_g[:, 1:2], in0=gmv[:, 0:1], in1=ab_g[:, 0:1])
            nc.vector.tensor_scalar_mul(out=ab_g[:, 1:2], in0=ab_g[:, 1:2],
                                        scalar1=-1.0)

            h_bf = []
            for ct in range(2):
                ab_ps = ps.tile([128, 2], fp32, tag="m1")
                nc.tensor.matmul(ab_ps, lhsT=q2[ct], rhs=ab_g, start=True, stop=True)
                AB = spool.tile([128, 2], fp32, tag=f"AB{ct}")
                nc.vector.tensor_mul(out=AB[:, 0:1], in0=ab_ps[:, 0:1], in1=gns_sb[ct])
                nc.vector.tensor_mul(out=AB[:, 1:2], in0=ab_ps[:, 1:2], in1=gns_sb[ct])
                nc.vector.tensor_add(out=AB[:, 1:2], in0=AB[:, 1:2], in1=gnb_sb[ct])
                ht = hpool.tile([128, S], bf16, tag=f"h{ct}")
                nc.vector.tensor_scalar(out=ht, in0=x_sb[ct],
                                        scalar1=AB[:, 0:1], scalar2=AB[:, 1:2],
                                        op0=OP.mult, op1=OP.add)
                if debug_taps and b == 0:
                    nc.sync.dma_start(out=dbg["h"][ct], in_=ht)
                    nc.sync.dma_start(out=dbg["ab"][ct], in_=AB)
                h_bf.append(ht)
            # residual tile absorbs b3 (x + b3 + W3 hh is the final output)
            for ct in range(2):
                nc.vector.tensor_scalar_add(out=x_sb[ct], in0=x_sb[ct],
                                            scalar1=b3_sb[ct])

            # q/k projections -> bf16 [d_tile 128, s 1024]
            qk_sb = [[None, None], [None, None]]
            for p, bias in ((0, b0_sb), (1, b1_sb)):
                for dt in range(2):
                    t = hpool.tile([128, S], bf16, tag=f"qk{p}{dt}")
                    for sc in range(2):
                        qk_ps = ps.tile([128, 512], fp32, tag=f"m{sc}",
                                        name="qk_ps")
                        for ct in range(2):
                            nc.tensor.matmul(
                                qk_ps,
                                lhsT=Wsb[p][ct][:, ts(dt, 128)],
                                rhs=h_bf[ct][:, ts(sc, 512)],
                                start=(ct == 0), stop=(ct == 1))
                        nc.vector.tensor_scalar_add(out=t[:, ts(sc, 512)],
                                                    in0=qk_ps, scalar1=bias[dt])
                    if debug_taps and b == 0:
                        nc.sync.dma_start(out=dbg["q" if p == 0 else "k"][dt], in_=t)
                    qk_sb[p][dt] = t
            qk_all.append(qk_sb)

            # v, produced transposed: vT[t, d] with ones column per head
            vt_tiles = []
            for j in range(8):
                vt_ps = ps.tile([128, C], fp32, tag=f"m{j % 2}", name="vt_ps")
                for ct in range(2):
                    nc.tensor.matmul(vt_ps, lhsT=h_bf[ct][:, ts(j, 128)],
                                     rhs=Wsb[2][ct], start=(ct == 0), stop=(ct == 1))
                vt = vpool.tile([128, NH, CH + 1], bf16, tag="vt")
                nc.gpsimd.memset(vt[:, :, CH:CH + 1], 1.0)
                nc.vector.tensor_tensor(
                    out=vt[:, :, 0:CH],
                    in0=vt_ps.rearrange("p (h c) -> p h c", h=NH),
                    in1=b2b.rearrange("p (h c) -> p h c", h=NH),
                    op=OP.add)
                if debug_taps and b == 0 and j == 0:
                    nc.sync.dma_start(out=dbg["vt0"][:, :, :], in_=vt)
                vt_tiles.append(vt)
            vt_all.append(vt_tiles)

        # ---- phase 2 per batch: attention pairs + final nin ----
        for b in range(B_PER_CORE):
            x_sb, _ = xs[b]
            qk_sb = qk_all[b]
            vt_tiles = vt_all[b]
            hh_sb = [None, None]
            for pr in range(2):
                hh_us = []
                for hp in range(2):
                    u = rpool.tile([CH + 1, S], fp32, tag=f"hhu{hp}",
                                   name="hh_u")
                    hh_us.append(u)
                for sc in range(2):
                    hh_ps = [ps.tile([CH + 1, 512], fp32, tag=f"h{i}",
                                     name=f"hh_ps{i}") for i in range(2)]
                    for j in range(8):
                        for hp in range(2):
                            s_ps = ps.tile([128, 512], fp32, tag=f"s{hp}",
                                           bufs=2, name="s_ps")
                            nc.tensor.matmul(
                                s_ps,
                                lhsT=qk_sb[1][pr][ts(hp, CH), ts(j, 128)],
                                rhs=qk_sb[0][pr][ts(hp, CH), ts(sc, 512)],
                                start=True, stop=True)
                            et = epool.tile([128, 512], bf16, tag="e")
                            nc.scalar.activation(out=et, in_=s_ps,
                                                 func=AF.Exp, scale=SCALE)
                            if debug_taps and b == 0 and pr == 0 and j == 0 and hp == 0:
                                nc.sync.dma_start(out=dbg["e00"][:, ts(sc, 512)], in_=et)
                            nc.tensor.matmul(
                                hh_ps[hp],
                                lhsT=vt_tiles[j][:, 2 * pr + hp, :],
                                rhs=et,
                                start=(j == 0), stop=(j == 7))
                    for hp in range(2):
                        nc.vector.tensor_copy(out=hh_us[hp][:, ts(sc, 512)],
                                              in_=hh_ps[hp])
                # normalize from SBUF
                hh_t = hpool.tile([128, S], bf16, tag="hh", bufs=4)
                for hp in range(2):
                    hh_u = hh_us[hp]
                    rd2 = rpool.tile([CH + 1, S], fp32, tag="rd2", name="rd2")
                    nc.vector.reciprocal_approx_fast(out=rd2, in_=hh_u)
                    if debug_taps and b == 0 and pr == 0 and hp == 0:
                        nc.sync.dma_start(out=dbg["rd20"][:, :], in_=rd2[CH:CH + 1, :])
                    rdd = dpool.tile([1, S], fp32, tag="rdd")
                    nc.gpsimd.dma_start(out=rdd, in_=rd2[CH:CH + 1, :])
                    rdb = rpool.tile([CH, S], fp32, tag="rdb")
                    nc.gpsimd.dma_start(out=rdb, in_=rdd.to_broadcast([CH, S]))
                    nc.vector.tensor_mul(out=hh_t[ts(hp, CH), :],
                                         in0=hh_u[0:CH, :], in1=rdb)
                if debug_taps and b == 0 and pr == 0:
                    nc.sync.dma_start(out=dbg["hh0"][:, :], in_=hh_t)
                hh_sb[pr] = hh_t

            for dt in range(2):
                out_t = xpool.tile([128, S], fp32, tag=f"out{dt}")
                for sc in range(2):
                    fin_ps = ps.tile([128, 512], fp32, tag=f"m{sc}",
                                     name="fin_ps")
                    for ct in range(2):
                        nc.tensor.matmul(
                            fin_ps,
                            lhsT=Wsb[3][ct][:, ts(dt, 128)],
                            rhs=hh_sb[ct][:, ts(sc, 512)],
                            start=(ct == 0), stop=(ct == 1))
                    nc.vector.tensor_add(out=out_t[:, ts(sc, 512)], in0=fin_ps,
                                         in1=x_sb[dt][:, ts(sc, 512)])
                nc.gpsimd.dma_start(out=y_d[b, ts(dt, 128), :], in_=out_t)

    nc.finalize()
    return nc


def _in_maps(inputs):
    x = np.ascontiguousarray(np.asarray(inputs["x"], dtype=np.float32))
    B = x.shape[0]
    xr = x.reshape(B, C, S)
    shared = {k: np.ascontiguousarray(np.asarray(inputs[k], dtype=np.float32))
              for k in ("gn_scale", "gn_bias", "W0", "b0", "W1", "b1", "W2", "b2",
                        "W3", "b3")}
    maps = []
    for core in range(N_CORES):
        m = dict(shared)
        m["x"] = np.ascontiguousarray(xr[core * B_PER_CORE:(core + 1) * B_PER_CORE])
        maps.append(m)
    return maps


def kernel(**inputs: np.ndarray) -> np.ndarray:
    from concourse.bass_utils import run_bass_kernel_spmd

    if "nc" not in _CACHE:
        _CACHE["nc"] = _build_nc()
    res = run_bass_kernel_spmd(_CACHE["nc"], _in_maps(inputs),
                               core_ids=list(range(N_CORES)))
    out = np.concatenate([res.results[c]["y"] for c in range(N_CORES)], axis=0)
    B = np.asarray(inputs["x"]).shape[0]
    return out.reshape(B, C, H, H).astype(np.float32)


def run_profiled(inputs):
    """Like kernel() but with trace=True; returns (out, exec_time_ns)."""
    from concourse.bass_utils import run_bass_kernel_spmd

    if "nc" not in _CACHE:
        _CACHE["nc"] = _build_nc()
    res = run_bass_kernel_spmd(_CACHE["nc"], _in_maps(inputs),
                               core_ids=list(range(N_CORES)), trace=True)
    out = np.concatenate([res.results[c]["y"] for c in range(N_CORES)], axis=0)
    B = np.asarray(inputs["x"]).shape[0]
    return out.reshape(B, C, H, H).astype(np.float32), res.exec_time_ns


# revision 33
# speedup vs baseline: 1.1422x; 1.0325x over previous
"""Trainium2 Bass kernel for nn_AttnBlockpp3d_old (GroupNorm + 4-head spatial
self-attention + residual), data-parallel over batch across 8 NeuronCores.

Shapes (hardcoded): x [16, 256, 32, 32] f32, 4 nin weights [256, 256] + biases,
gn scale/bias [256]. Each core processes 2 batches of [256, 1024].

Structure (per core): phase 1 runs GroupNorm stats + q/k/v projections for
BOTH batches up front; phase 2 runs the attention pairs + final nin
back-to-back so the ScalarE softmax-exp stream (the bottleneck engine) is
continuous and the PE stays HAM-warm.

Key tricks:
- GroupNorm stats via bn_stats on a contiguous [128, 2048] view (partition
  4g+j holds 2 channels of group g); group-combine and channel-broadcast via
  tiny indicator matmuls; rsqrt as exp(-0.5*ln(var+eps)) so ScalarE stays on
  the ln+exp table set used by the softmax.
- v is produced directly transposed (h slices stationary, W2 moving): no PE
  transposes anywhere.
- Scores are computed transposed ([t, s], k stationary) with two heads packed
  into the PE array via row tiling (64-partition contraction each).
- Softmax exp on ScalarE straight out of PSUM with the 1/sqrt(64) scale
  folded into the activation; no max-subtraction (scores are O(+-7)).
- The softmax denominator rides the A@V matmul as a ones-column in the
  stationary operand; normalization = reciprocal_approx_fast + DRAM-bounce
  partition-broadcast DMA, fused into the mandatory PSUM->SBUF move.
- Final nin adds b3 via a K=1 matmul; the residual rides the PSUM->SBUF move.
"""

import numpy as np

N_CORES = 8
B_TOTAL = 16
B_PER_CORE = B_TOTAL // N_CORES
C = 256
H = 32
S = H * H          # 1024 spatial positions (N_FRAMES=1)
NG = 32            # groupnorm groups -> 8 channels/group
NH = 4             # heads
CH = C // NH       # 64 channels/head
EPS = 1e-6
SCALE = CH ** -0.5  # 0.125

_CACHE: dict = {}


def _build_nc(debug_taps=False):
    from contextlib import ExitStack

    import concourse.bacc as bacc
    import concourse.bass as bass
    import concourse.mybir as mybir
    import concourse.tile as tile

    fp32 = mybir.dt.float32
    bf16 = mybir.dt.bfloat16
    AF = mybir.ActivationFunctionType
    OP = mybir.AluOpType
    ts = bass.ts

    nc = bacc.Bacc("TRN2")

    x_d = nc.dram_tensor("x", [B_PER_CORE, C, S], fp32, kind="ExternalInput")
    gns_d = nc.dram_tensor("gn_scale", [C], fp32, kind="ExternalInput")
    gnb_d = nc.dram_tensor("gn_bias", [C], fp32, kind="ExternalInput")
    W_d = [nc.dram_tensor(f"W{i}", [C, C], fp32, kind="ExternalInput") for i in range(4)]
    b_d = [nc.dram_tensor(f"b{i}", [C], fp32, kind="ExternalInput") for i in range(4)]
    y_d = nc.dram_tensor("y", [B_PER_CORE, C, S], fp32, kind="ExternalOutput")
    dbg = {}
    if debug_taps:
        for nm, shp, dt_ in (("h", [2, 128, S], bf16), ("q", [2, 128, S], bf16),
                             ("k", [2, 128, S], bf16), ("vt0", [128, NH, CH + 1], bf16),
                             ("e00", [128, S], bf16), ("rd20", [1, S], fp32),
                             ("hh0", [128, S], bf16), ("ab", [2, 128, 2], fp32)):
            dbg[nm] = nc.dram_tensor(f"dbg_{nm}", shp, dt_, kind="ExternalOutput")

    with tile.TileContext(nc) as tc, ExitStack() as ctx:
        const = ctx.enter_context(tc.tile_pool(name="const", bufs=1))
        stage = ctx.enter_context(tc.tile_pool(name="stage", bufs=2))
        xpool = ctx.enter_context(tc.tile_pool(name="xpool", bufs=2))
        hpool = ctx.enter_context(tc.tile_pool(name="hpool", bufs=2))
        vpool = ctx.enter_context(tc.tile_pool(name="vpool", bufs=18))
        epool = ctx.enter_context(tc.tile_pool(name="epool", bufs=6))
        rpool = ctx.enter_context(tc.tile_pool(name="rpool", bufs=2))
        dpool = ctx.enter_context(tc.tile_pool(name="dpool", bufs=4, space="DRAM"))
        spool = ctx.enter_context(tc.tile_pool(name="spool", bufs=3))

        # PSUM (8 banks): T0/T1 = 2-bank slots (hh accumulators / qkv / fin),
        # s0/s1 = 1-bank slots x2 bufs (scores double-buffer / vt / stats).
        ps = ctx.enter_context(tc.tile_pool(name="ps", bufs=1, space="PSUM"))

        # ---- phase 0: loads + constants ----
        # x loads first (stats are on the critical path)
        xs = []
        for b in range(B_PER_CORE):
            xg = xpool.tile([128, 2 * S], fp32, tag="xg")
            nc.sync.dma_start(out=xg, in_=x_d[b].rearrange("(p a) s -> p (a s)", p=128))
            x_sb = []
            for ct in range(2):
                t = xpool.tile([128, S], fp32, tag=f"x{b}{ct}", name=f"x_sb{b}{ct}")
                nc.sync.dma_start(out=t, in_=x_d[b, ts(ct, 128), :])
                x_sb.append(t)
            xs.append((x_sb, xg))

        # W0..W3 as bf16 [128, c_tile 2, d 256] (partition p = channel p + 128*ct)
        Wsb_t = []
        for i in range(4):
            st = stage.tile([128, 2, C], fp32, tag="wstage")
            nc.sync.dma_start(out=st, in_=W_d[i].rearrange("(a p) d -> p a d", p=128))
            wt = const.tile([128, 2, C], bf16, tag=f"w{i}")
            nc.gpsimd.tensor_copy(out=wt, in_=st)
            Wsb_t.append(wt)
        Wsb = [[Wsb_t[i][:, ct, :] for ct in range(2)] for i in range(4)]

        def col_tiles(dram, name):
            out = []
            for ct in range(2):
                t = const.tile([128, 1], fp32, tag=f"{name}{ct}")
                nc.sync.dma_start(out=t, in_=dram[ts(ct, 128)][:, None])
                out.append(t)
            return out

        gns_sb = col_tiles(gns_d, "gns")
        gnb_sb = col_tiles(gnb_d, "gnb")
        b0_sb = col_tiles(b_d[0], "b0")
        b1_sb = col_tiles(b_d[1], "b1")

        b2b = const.tile([128, C], fp32, tag="b2b")
        nc.sync.dma_start(out=b2b, in_=b_d[2][None, :].to_broadcast([128, C]))

        b3_sb = col_tiles(b_d[3], "b3")

        eps_t = const.tile([32, 1], fp32, tag="eps")
        nc.vector.memset(eps_t, EPS)

        # HAM warm-up: dummy matmuls with no data deps keep the PE busy during
        # the load phase so real matmuls start at the unthrottled clock.
        warm = const.tile([128, 512], bf16, tag="warm")
        nc.vector.memset(warm, 1.0)
        warm_ps = ps.tile([128, 512], fp32, tag="s0", bufs=2, name="warm_ps")
        for i in range(40):
            nc.tensor.matmul(warm_ps, lhsT=warm[:, 0:128], rhs=warm,
                             start=True, stop=True)

        # Q1 [128, 32]: Q1[p, g] = 1 iff p//4 == g   (stats partition -> group)
        q1 = const.tile([128, NG], fp32, tag="q1")
        nc.gpsimd.memset(q1, 1.0)
        nc.gpsimd.affine_select(out=q1, in_=q1, compare_op=OP.is_ge, fill=0.0,
                                pattern=[[-4, NG]], base=0, channel_multiplier=1)
        nc.gpsimd.affine_select(out=q1, in_=q1, compare_op=OP.is_ge, fill=0.0,
                                pattern=[[4, NG]], base=3, channel_multiplier=-1)

        # Q2[ct] [32, 128]: Q2[g, c] = 1 iff group(global_c) == g
        q2 = []
        for ct in range(2):
            t = const.tile([NG, 128], fp32, tag=f"q2{ct}")
            nc.gpsimd.memset(t, 1.0)
            base = ct * 128
            nc.gpsimd.affine_select(out=t, in_=t, compare_op=OP.is_ge, fill=0.0,
                                    pattern=[[1, 128]], base=base, channel_multiplier=-8)
            nc.gpsimd.affine_select(out=t, in_=t, compare_op=OP.is_ge, fill=0.0,
                                    pattern=[[-1, 128]], base=7 - base, channel_multiplier=8)
            q2.append(t)

        # ---- phase 1 per batch: stats + normalize + q/k/vT ----
        qk_all, vt_all = [], []
        for b in range(B_PER_CORE):
            x_sb, xg = xs[b]
            st6 = spool.tile([128, 4, 6], fp32, tag="st6")
            for i in range(4):
                nc.vector.bn_stats(out=st6[:, i, :], in_=xg[:, ts(i, 512)])
            mv = spool.tile([128, 2], fp32, tag="mv")
            nc.vector.bn_aggr(out=mv, in_=st6)
            rhs2 = spool.tile([128, 2], fp32, tag="rhs2")
            nc.vector.tensor_copy(out=rhs2[:, 0:1], in_=mv[:, 0:1])
            nc.vector.tensor_mul(out=rhs2[:, 1:2], in0=mv[:, 0:1], in1=mv[:, 0:1])
            nc.vector.tensor_add(out=rhs2[:, 1:2], in0=rhs2[:, 1:2], in1=mv[:, 1:2])
            gs_ps = ps.tile([NG, 2], fp32, tag="m0")
            nc.tensor.matmul(gs_ps, lhsT=q1, rhs=rhs2, start=True, stop=True)
            gmv = spool.tile([NG, 2], fp32, tag="gmv")
            nc.vector.tensor_scalar_mul(out=gmv, in0=gs_ps, scalar1=0.25)
            varg = spool.tile([NG, 1], fp32, tag="varg")
            nc.vector.tensor_mul(out=varg, in0=gmv[:, 0:1], in1=gmv[:, 0:1])
            nc.vector.tensor_tensor(out=varg, in0=gmv[:, 1:2], in1=varg,
                                    op=OP.subtract)
            ab_g = spool.tile([NG, 2], fp32, tag="abg")
            lnv = spool.tile([NG, 1], fp32, tag="lnv")
            nc.scalar.activation(out=lnv, in_=varg, func=AF.Ln, bias=eps_t, scale=1.0)
            nc.scalar.activation(out=ab_g[:, 0:1], in_=lnv, func=AF.Exp, scale=-0.5)
            nc.vector.tensor_mul(out=ab_g[:, 1:2], in0=gmv[:, 0:1], in1=ab_g[:, 0:1])
            nc.vector.tensor_scalar_mul(out=ab_g[:, 1:2], in0=ab_g[:, 1:2],
                                        scalar1=-1.0)

            h_bf = []
            for ct in range(2):
                ab_ps = ps.tile([128, 2], fp32, tag="m1")
                nc.tensor.matmul(ab_ps, lhsT=q2[ct], rhs=ab_g, start=True, stop=True)
                AB = spool.tile([128, 2], fp32, tag=f"AB{ct}")
                nc.vector.tensor_mul(out=AB[:, 0:1], in0=ab_ps[:, 0:1], in1=gns_sb[ct])
                nc.vector.tensor_mul(out=AB[:, 1:2], in0=ab_ps[:, 1:2], in1=gns_sb[ct])
                nc.vector.tensor_add(out=AB[:, 1:2], in0=AB[:, 1:2], in1=gnb_sb[ct])
                ht = hpool.tile([128, S], bf16, tag=f"h{ct}")
                nc.vector.tensor_scalar(out=ht, in0=x_sb[ct],
                                        scalar1=AB[:, 0:1], scalar2=AB[:, 1:2],
                                        op0=OP.mult, op1=OP.add)
                if debug_taps and b == 0:
                    nc.sync.dma_start(out=dbg["h"][ct], in_=ht)
                    nc.sync.dma_start(out=dbg["ab"][ct], in_=AB)
                h_bf.append(ht)
            # residual tile absorbs b3 (x + b3 + W3 hh is the final output)
            for ct in range(2):
                nc.vector.tensor_scalar_add(out=x_sb[ct], in0=x_sb[ct],
                                            scalar1=b3_sb[ct])

            # q/k projections -> bf16 [d_tile 128, s 1024]
            qk_sb = [[None, None], [None, None]]
            vt_tiles = []
            for dt in range(2):
                for p, bias in ((0, b0_sb), (1, b1_sb)):
                    t = hpool.tile([128, S], bf16, tag=f"qk{p}{dt}")
                    for sc in range(2):
                        qk_ps = ps.tile([128, 512], fp32, tag=f"m{sc}",
                                        name="qk_ps")
                        for ct in range(2):
                            nc.tensor.matmul(
                                qk_ps,
                                lhsT=Wsb[p][ct][:, ts(dt, 128)],
                                rhs=h_bf[ct][:, ts(sc, 512)],
                                start=(ct == 0), stop=(ct == 1))
                        if b == 0:
                            # ScalarE is idle before the softmax stream starts;
                            # only batch 0's copies may ride it (later-data ops
                            # would head-of-line-block the exps)
                            nc.scalar.activation(out=t[:, ts(sc, 512)],
                                                 in_=qk_ps, func=AF.Identity,
                                                 bias=bias[dt], scale=1.0)
                        else:
                            nc.vector.tensor_scalar_add(out=t[:, ts(sc, 512)],
                                                        in0=qk_ps,
                                                        scalar1=bias[dt])
                    if debug_taps and b == 0:
                        nc.sync.dma_start(out=dbg["q" if p == 0 else "k"][dt], in_=t)
                    qk_sb[p][dt] = t
                if dt == 0:
                    # vT right after the d-tile-0 projections so pair 0's
                    # attention has everything it needs as early as possible
                    for j in range(8):
                        vt_ps = ps.tile([128, C], fp32, tag=f"m{j % 2}", name="vt_ps")
                        for ct in range(2):
                            nc.tensor.matmul(vt_ps, lhsT=h_bf[ct][:, ts(j, 128)],
                                             rhs=Wsb[2][ct], start=(ct == 0),
                                             stop=(ct == 1))
                        vt = vpool.tile([128, NH, CH + 1], bf16, tag="vt")
                        nc.gpsimd.memset(vt[:, :, CH:CH + 1], 1.0)
                        nc.vector.tensor_tensor(
                            out=vt[:, :, 0:CH],
                            in0=vt_ps.rearrange("p (h c) -> p h c", h=NH),
                            in1=b2b.rearrange("p (h c) -> p h c", h=NH),
                            op=OP.add)
                        if debug_taps and b == 0 and j == 0:
                            nc.sync.dma_start(out=dbg["vt0"][:, :, :], in_=vt)
                        vt_tiles.append(vt)
            qk_all.append(qk_sb)
            vt_all.append(vt_tiles)

        # ---- phase 2 per batch: attention pairs + final nin ----
        for b in range(B_PER_CORE):
            x_sb, _ = xs[b]
            qk_sb = qk_all[b]
            vt_tiles = vt_all[b]
            hh_sb = [None, None]
            for pr in range(2):
                hh_us = []
                for hp in range(2):
                    u = rpool.tile([CH + 1, S], fp32, tag=f"hhu{hp}",
                                   name="hh_u")
                    hh_us.append(u)
                for sc in range(2):
                    hh_ps = [ps.tile([CH + 1, 512], fp32, tag=f"h{i}",
                                     name=f"hh_ps{i}") for i in range(2)]
                    for j in range(8):
                        for hp in range(2):
                            s_ps = ps.tile([128, 512], fp32, tag=f"s{hp}",
                                           bufs=2, name="s_ps")
                            nc.tensor.matmul(
                                s_ps,
                                lhsT=qk_sb[1][pr][ts(hp, CH), ts(j, 128)],
                                rhs=qk_sb[0][pr][ts(hp, CH), ts(sc, 512)],
                                start=True, stop=True)
                            et = epool.tile([128, 512], bf16, tag="e")
                            nc.scalar.activation(out=et, in_=s_ps,
                                                 func=AF.Exp, scale=SCALE)
                            if debug_taps and b == 0 and pr == 0 and j == 0 and hp == 0:
                                nc.sync.dma_start(out=dbg["e00"][:, ts(sc, 512)], in_=et)
                            nc.tensor.matmul(
                                hh_ps[hp],
                                lhsT=vt_tiles[j][:, 2 * pr + hp, :],
                                rhs=et,
                                start=(j == 0), stop=(j == 7))
                    for hp in range(2):
                        nc.vector.tensor_copy(out=hh_us[hp][:, ts(sc, 512)],
                                              in_=hh_ps[hp])
                # normalize from SBUF
                hh_t = hpool.tile([128, S], bf16, tag="hh", bufs=4)
                for hp in range(2):
                    hh_u = hh_us[hp]
                    rd2 = rpool.tile([CH + 1, S], fp32, tag="rd2", name="rd2")
                    nc.vector.reciprocal_approx_fast(out=rd2, in_=hh_u)
                    if debug_taps and b == 0 and pr == 0 and hp == 0:
                        nc.sync.dma_start(out=dbg["rd20"][:, :], in_=rd2[CH:CH + 1, :])
                    rdd = dpool.tile([1, S], fp32, tag="rdd")
                    nc.gpsimd.dma_start(out=rdd, in_=rd2[CH:CH + 1, :])
                    rdb = rpool.tile([CH, S], fp32, tag="rdb")
                    nc.gpsimd.dma_start(out=rdb, in_=rdd.to_broadcast([CH, S]))
                    nc.vector.tensor_mul(out=hh_t[ts(hp, CH), :],
                                         in0=hh_u[0:CH, :], in1=rdb)
                if debug_taps and b == 0 and pr == 0:
                    nc.sync.dma_start(out=dbg["hh0"][:, :], in_=hh_t)
                hh_sb[pr] = hh_t

            for dt in range(2):
                out_t = xpool.tile([128, S], fp32, tag=f"out{dt}")
                for sc in range(2):
                    fin_ps = ps.tile([128, 512], fp32, tag=f"m{sc}",
                                     name="fin_ps")
                    for ct in range(2):
                        nc.tensor.matmul(
                            fin_ps,
                            lhsT=Wsb[3][ct][:, ts(dt, 128)],
                            rhs=hh_sb[ct][:, ts(sc, 512)],
                            start=(ct == 0), stop=(ct == 1))
                    nc.vector.tensor_add(out=out_t[:, ts(sc, 512)], in0=fin_ps,
                                         in1=x_sb[dt][:, ts(sc, 512)])
                nc.gpsimd.dma_start(out=y_d[b, ts(dt, 128), :], in_=out_t)

    nc.finalize()
    return nc


def _in_maps(inputs):
    x = np.ascontiguousarray(np.asarray(inputs["x"], dtype=np.float32))
    B = x.shape[0]
    xr = x.reshape(B, C, S)
    shared = {k: np.ascontiguousarray(np.asarray(inputs[k], dtype=np.float32))
              for k in ("gn_scale", "gn_bias", "W0", "b0", "W1", "b1", "W2", "b2",
                        "W3", "b3")}
    maps = []
    for core in range(N_CORES):
        m = dict(shared)
        m["x"] = np.ascontiguousarray(xr[core * B_PER_CORE:(core + 1) * B_PER_CORE])
        maps.append(m)
    return maps


def kernel(**inputs: np.ndarray) -> np.ndarray:
    from concourse.bass_utils import run_bass_kernel_spmd

    if "nc" not in _CACHE:
        _CACHE["nc"] = _build_nc()
    res = run_bass_kernel_spmd(_CACHE["nc"], _in_maps(inputs),
                               core_ids=list(range(N_CORES)))
    out = np.concatenate([res.results[c]["y"] for c in range(N_CORES)], axis=0)
    B = np.asarray(inputs["x"]).shape[0]
    return out.reshape(B, C, H, H).astype(np.float32)


def run_profiled(inputs):
    """Like kernel() but with trace=True; returns (out, exec_time_ns)."""
    from concourse.bass_utils import run_bass_kernel_spmd

    if "nc" not in _CACHE:
        _CACHE["nc"] = _build_nc()
    res = run_bass_kernel_spmd(_CACHE["nc"], _in_maps(inputs),
                               core_ids=list(range(N_CORES)), trace=True)
    out = np.concatenate([res.results[c]["y"] for c in range(N_CORES)], axis=0)
    B = np.asarray(inputs["x"]).shape[0]
    return out.reshape(B, C, H, H).astype(np.float32), res.exec_time_ns
